# revision 17
# baseline (speedup 1.0000x reference)
"""Trainium2 Bass kernel for the Conv-RBS density-matrix problem.

Math: the reference applies 18 RBS (Givens) gates sequentially as
rho <- U rho U^T.  Conjugations compose, and every gate factorizes over
the (row=16, col=16, ch=4) tensor-product structure of the 1024-dim
space, so the whole scan collapses to

    out = W @ rho @ W.T,   W = R (x) C (x) H

with R, C 16x16 rotations that are identity outside their top-left 4x4
block, and H a dense 4x4 rotation.  In 128x128 blocks W is
block-diagonal with B = I2 (x) C (x) H everywhere except a 2x2 block
grid in the top-left 256x256 corner (R's 4x4 block).

Device scheme (per core c of 8, column-sharded on rho):
  pass A:  ZT[c,I] = sum_K  rho[K,c]^T @ W[I,K]^T     (TensorE: lhsT=rho
           block, rhs=const) -- i.e. the c-th block-row of (W rho)^T.
  pass B:  out[I,J] += ZT[c,I]^T @ W[J,c]^T           (lhsT=ZT block)
Cores c>=2 produce the finished column slab out[:,c]; cores 0,1 produce
partial sums for columns 0..255 which the host adds.  No transposes and
no collectives are needed anywhere.
"""

import contextlib
import ctypes
import os
import sys
import types

import numpy as np

import concourse.bass as bass
import concourse.mybir as mybir
from concourse import bacc, bass_utils
from concourse.tile import TileContext


def _install_axon_ntff_shim():
    """The agent image's ``antenv`` lacks ``axon_hooks``, so bass_utils'
    trace=True path crashes instead of profiling.  Recreate the hook the
    way trn_boot would have: ctypes into libaxon_pjrt.so."""
    try:
        import antenv.axon_hooks  # noqa: F401
        return
    except ImportError:
        pass
    so_path = "/opt/axon/libaxon_pjrt.so"
    hook = None
    if os.path.exists(so_path):
        try:
            lib = ctypes.CDLL(so_path)
            lib.axon_start_nrt_profile.argtypes = [
                ctypes.POINTER(ctypes.c_int64),
                ctypes.c_size_t,
            ]
            lib.axon_start_nrt_profile.restype = ctypes.c_int64
            lib.axon_stop_nrt_profile.argtypes = [ctypes.c_char_p]
            lib.axon_stop_nrt_profile.restype = ctypes.c_int64

            @contextlib.contextmanager
            def _hook(output_dir, device_ids):
                import jax

                jax.devices()
                if device_ids:
                    ids = (ctypes.c_int64 * len(device_ids))(*device_ids)
                    rc = lib.axon_start_nrt_profile(ids, len(device_ids))
                else:
                    rc = lib.axon_start_nrt_profile(None, 0)
                if rc != 0:
                    raise RuntimeError(f"axon_start_nrt_profile rc={rc}")
                try:
                    yield
                finally:
                    n = lib.axon_stop_nrt_profile(str(output_dir).encode())
                    if n < 0:
                        raise RuntimeError(f"axon_stop_nrt_profile rc={n}")

            hook = _hook
        except (OSError, AttributeError):
            hook = None

    mod = types.ModuleType("antenv.axon_hooks")
    mod.get_axon_ntff_profile_hook = lambda: hook
    mod.set_axon_ntff_profile_hook = lambda h: None
    sys.modules["antenv.axon_hooks"] = mod


def _patch_upload_artifacts():
    """Artifact upload needs bucket creds this container may not have;
    don't let a failed upload kill the profiled run."""
    orig = bass_utils.upload_artifacts
    if getattr(orig, "_safe_wrapped", False):
        return

    def safe_upload(tmpdir):
        try:
            return orig(tmpdir)
        except Exception:
            return tmpdir

    safe_upload._safe_wrapped = True
    bass_utils.upload_artifacts = safe_upload


_install_axon_ntff_shim()
_patch_upload_artifacts()

I_DIM, J_DIM, KGATE = 16, 4, 4
D = 1024
P = 128
NCORES = 8

LAST_EXEC_NS = None  # filled when BASS_TRACE is set


def _gate_list():
    gates = []
    for i in range(KGATE):
        for j in range(i + 1, KGATE):
            gates.append(("row", i, j))
    for i in range(KGATE):
        for j in range(i + 1, KGATE):
            gates.append(("col", i, j))
    for i in range(J_DIM):
        for j in range(i + 1, J_DIM):
            gates.append(("ch", i, j))
    return gates


def _build_w_blocks(thetas):
    """Return (BT, PA0, PA1, RHSB[8]) fp32 host constants."""
    mats = {"row": np.eye(I_DIM), "col": np.eye(I_DIM), "ch": np.eye(J_DIM)}
    for (reg, a, b), th in zip(_gate_list(), np.asarray(thetas, dtype=np.float64)):
        n = mats[reg].shape[0]
        G = np.eye(n)
        c, s = np.cos(th), np.sin(th)
        G[a, a] = c
        G[b, b] = c
        G[a, b] = s
        G[b, a] = -s
        mats[reg] = G @ mats[reg]
    R, C, H = mats["row"], mats["col"], mats["ch"]
    Q = np.kron(C, H)  # 64x64
    B = np.kron(np.eye(2), Q)  # 128x128, W[I,I] for I>=2
    # top-left 2x2 block grid: W[i,k] = R[2i:2i+2, 2k:2k+2] (x) Q
    Wtop = [[np.kron(R[2 * i : 2 * i + 2, 2 * k : 2 * k + 2], Q) for k in range(2)]
            for i in range(2)]
    BT = B.T
    # pass-A packed rhs for I<2:  PA[K] = [ W[0,K]^T | W[1,K]^T ]
    PA0 = np.concatenate([Wtop[0][0].T, Wtop[1][0].T], axis=1)
    PA1 = np.concatenate([Wtop[0][1].T, Wtop[1][1].T], axis=1)
    # pass-B rhs per core:  c<2 -> [ W[0,c]^T | W[1,c]^T ],  c>=2 -> [ B^T | 0 ]
    rhsb = []
    for c in range(NCORES):
        if c < 2:
            rhsb.append(np.concatenate([Wtop[0][c].T, Wtop[1][c].T], axis=1))
        else:
            rhsb.append(np.concatenate([BT, np.zeros_like(BT)], axis=1))
    f32 = np.float32
    return (
        np.ascontiguousarray(BT, dtype=f32),
        np.ascontiguousarray(PA0, dtype=f32),
        np.ascontiguousarray(PA1, dtype=f32),
        [np.ascontiguousarray(r, dtype=f32) for r in rhsb],
    )


# blob columns: BT | RB | rho2..rho7 | PA0 | PA1 | rho0 rho1 — ordered by
# when the kernel needs them; the input DMA is split into a serialized
# chain along this order so early chunks land early (concurrent DMAs
# round-robin at packet granularity and would all finish together)
BLOB_W = 1920
_BT_C, _RB_C, _RHO27_C, _PA_C, _RHO01_C = 0, 128, 384, 1152, 1664
# chain chunk boundaries (columns)
_CHAIN = [0, 384, 640, 896, 1152, 1664, 1920]


def _build_program():
    f32 = mybir.dt.float32
    nc = bacc.Bacc(None)
    blob_in = nc.declare_dram_parameter("blob", [P, BLOB_W], f32, isOutput=False)
    # partition-major output: outp[p, I*256+n] = out[I*128+p, n] -- keeps
    # DMA descriptors at 8KB instead of 1KB; host un-transposes
    outp = nc.declare_dram_parameter("outp", [P, 8 * 256], f32, isOutput=True)

    from concourse.tile_rust import add_dep_helper

    with TileContext(nc) as tc:
        with (
            tc.tile_pool(name="const", bufs=1) as cpool,
            tc.tile_pool(name="psum", bufs=2, space="PSUM") as ppool,
        ):
            blob = cpool.tile([P, BLOB_W], f32)
            prev = None
            for lo, hi in zip(_CHAIN[:-1], _CHAIN[1:]):
                d = nc.sync.dma_start(blob[:, lo:hi], blob_in[:, lo:hi])
                di = d.ins if hasattr(d, "ins") else d
                if prev is not None:
                    add_dep_helper(di, prev, reason="serialize input DMA chain")
                prev = di

            zt = cpool.tile([P, 8, P], f32)
            rho = {}
            for pos, K in enumerate([2, 3, 4, 5, 6, 7]):
                rho[K] = blob[:, _RHO27_C + pos * P : _RHO27_C + (pos + 1) * P]
            rho[0] = blob[:, _RHO01_C : _RHO01_C + P]
            rho[1] = blob[:, _RHO01_C + P : _RHO01_C + 2 * P]
            BT = blob[:, _BT_C : _BT_C + 128]
            PA = [blob[:, _PA_C : _PA_C + 256], blob[:, _PA_C + 256 : _PA_C + 512]]
            rb = blob[:, _RB_C : _RB_C + 256]

            def pass_b(I):
                ps = ppool.tile([P, 256], f32, tag="psB")
                nc.tensor.matmul(ps[:], zt[:, I, :], rb, start=True, stop=True)
                ob = cpool.tile([P, 256], f32, tag=f"ob{I}")
                nc.vector.tensor_copy(out=ob[:], in_=ps[:])
                nc.sync.dma_start(outp[:, I * 256 : (I + 1) * 256], ob[:])

            # interleave pass A / pass B per block: B(I) only needs ZT[c,I]
            for I in range(2, 8):
                ps = ppool.tile([P, P], f32, tag="psA")
                nc.tensor.matmul(ps[:], rho[I], BT, start=True, stop=True)
                nc.vector.tensor_copy(out=zt[:, I, :], in_=ps[:])
                pass_b(I)
            ps2 = ppool.tile([P, 256], f32, tag="psA2")
            nc.tensor.matmul(ps2[:], rho[0], PA[0], start=True, stop=False)
            nc.tensor.matmul(ps2[:], rho[1], PA[1], start=False, stop=True)
            nc.vector.tensor_copy(out=zt[:, 0, :], in_=ps2[:, 0:128])
            nc.vector.tensor_copy(out=zt[:, 1, :], in_=ps2[:, 128:256])
            pass_b(0)
            pass_b(1)
    if not nc.is_finalized():
        nc.finalize()
    return nc


def _run(inputs, trace=False):
    global LAST_EXEC_NS
    rho = np.ascontiguousarray(np.asarray(inputs["input_state"], dtype=np.float32))
    thetas = np.asarray(inputs["thetas"], dtype=np.float32)
    BT, PA0, PA1, rhsb = _build_w_blocks(thetas)

    nc = _build_program()
    in_maps = []
    for c in range(NCORES):
        slab = rho[:, c * P : (c + 1) * P].reshape(8, P, P)
        km = lambda ks: np.transpose(slab[ks], (1, 0, 2)).reshape(P, -1)
        # layout: BT | RB | rho2..7 | PA0 PA1 | rho0 rho1
        blob = np.concatenate(
            [BT, rhsb[c], km([2, 3, 4, 5, 6, 7]), PA0, PA1, km([0, 1])], axis=1
        )
        in_maps.append({"blob": np.ascontiguousarray(blob)})
    res = bass_utils.run_bass_kernel_spmd(
        nc, in_maps, list(range(NCORES)), trace=trace
    )
    LAST_EXEC_NS = res.exec_time_ns

    out = np.empty((D, D), dtype=np.float32)

    def unpack(c):
        # outp[p, I*256+n] -> [1024, 256]
        return (
            res.results[c]["outp"]
            .reshape(P, 8, 256)
            .transpose(1, 0, 2)
            .reshape(D, 256)
        )

    for c in range(2, NCORES):
        out[:, c * P : (c + 1) * P] = unpack(c)[:, :P]
    out[:, 0:256] = unpack(0) + unpack(1)
    return out


def kernel(**inputs):
    return _run(inputs)


# revision 19
# speedup vs baseline: 1.2771x; 1.2771x over previous
"""Trainium2 Bass kernel for the Conv-RBS density-matrix problem.

Math: the reference applies 18 RBS (Givens) gates sequentially as
rho <- U rho U^T.  Conjugations compose, and every gate factorizes over
the (row=16, col=16, ch=4) tensor-product structure of the 1024-dim
space, so the whole scan collapses to

    out = W @ rho @ W.T,   W = R (x) C (x) H

with R, C 16x16 rotations that are identity outside their top-left 4x4
block, and H a dense 4x4 rotation.  In 128x128 blocks W is
block-diagonal with B = I2 (x) C (x) H everywhere except a 2x2 block
grid in the top-left 256x256 corner (R's 4x4 block).

Device scheme (per core c of 8, column-sharded on rho):
  pass A:  ZT[c,I] = sum_K  rho[K,c]^T @ W[I,K]^T     (TensorE: lhsT=rho
           block, rhs=const) -- i.e. the c-th block-row of (W rho)^T.
  pass B:  out[I,J] += ZT[c,I]^T @ W[J,c]^T           (lhsT=ZT block)
Cores c>=2 produce the finished column slab out[:,c]; cores 0,1 produce
partial sums for columns 0..255 which the host adds.  No transposes and
no collectives are needed anywhere.
"""

import contextlib
import ctypes
import os
import sys
import types

import numpy as np

import concourse.bass as bass
import concourse.mybir as mybir
from concourse import bacc, bass_utils
from concourse.tile import TileContext


def _install_axon_ntff_shim():
    """The agent image's ``antenv`` lacks ``axon_hooks``, so bass_utils'
    trace=True path crashes instead of profiling.  Recreate the hook the
    way trn_boot would have: ctypes into libaxon_pjrt.so."""
    try:
        import antenv.axon_hooks  # noqa: F401
        return
    except ImportError:
        pass
    so_path = "/opt/axon/libaxon_pjrt.so"
    hook = None
    if os.path.exists(so_path):
        try:
            lib = ctypes.CDLL(so_path)
            lib.axon_start_nrt_profile.argtypes = [
                ctypes.POINTER(ctypes.c_int64),
                ctypes.c_size_t,
            ]
            lib.axon_start_nrt_profile.restype = ctypes.c_int64
            lib.axon_stop_nrt_profile.argtypes = [ctypes.c_char_p]
            lib.axon_stop_nrt_profile.restype = ctypes.c_int64

            @contextlib.contextmanager
            def _hook(output_dir, device_ids):
                import jax

                jax.devices()
                if device_ids:
                    ids = (ctypes.c_int64 * len(device_ids))(*device_ids)
                    rc = lib.axon_start_nrt_profile(ids, len(device_ids))
                else:
                    rc = lib.axon_start_nrt_profile(None, 0)
                if rc != 0:
                    raise RuntimeError(f"axon_start_nrt_profile rc={rc}")
                try:
                    yield
                finally:
                    n = lib.axon_stop_nrt_profile(str(output_dir).encode())
                    if n < 0:
                        raise RuntimeError(f"axon_stop_nrt_profile rc={n}")

            hook = _hook
        except (OSError, AttributeError):
            hook = None

    mod = types.ModuleType("antenv.axon_hooks")
    mod.get_axon_ntff_profile_hook = lambda: hook
    mod.set_axon_ntff_profile_hook = lambda h: None
    sys.modules["antenv.axon_hooks"] = mod


def _patch_upload_artifacts():
    """Artifact upload needs bucket creds this container may not have;
    don't let a failed upload kill the profiled run."""
    orig = bass_utils.upload_artifacts
    if getattr(orig, "_safe_wrapped", False):
        return

    def safe_upload(tmpdir):
        try:
            return orig(tmpdir)
        except Exception:
            return tmpdir

    safe_upload._safe_wrapped = True
    bass_utils.upload_artifacts = safe_upload


_install_axon_ntff_shim()
_patch_upload_artifacts()

I_DIM, J_DIM, KGATE = 16, 4, 4
D = 1024
P = 128
NCORES = 8

LAST_EXEC_NS = None  # filled when BASS_TRACE is set


def _gate_list():
    gates = []
    for i in range(KGATE):
        for j in range(i + 1, KGATE):
            gates.append(("row", i, j))
    for i in range(KGATE):
        for j in range(i + 1, KGATE):
            gates.append(("col", i, j))
    for i in range(J_DIM):
        for j in range(i + 1, J_DIM):
            gates.append(("ch", i, j))
    return gates


def _build_w_blocks(thetas):
    """Return (BT, PA0, PA1, RHSB[8]) fp32 host constants."""
    mats = {"row": np.eye(I_DIM), "col": np.eye(I_DIM), "ch": np.eye(J_DIM)}
    for (reg, a, b), th in zip(_gate_list(), np.asarray(thetas, dtype=np.float64)):
        n = mats[reg].shape[0]
        G = np.eye(n)
        c, s = np.cos(th), np.sin(th)
        G[a, a] = c
        G[b, b] = c
        G[a, b] = s
        G[b, a] = -s
        mats[reg] = G @ mats[reg]
    R, C, H = mats["row"], mats["col"], mats["ch"]
    Q = np.kron(C, H)  # 64x64
    B = np.kron(np.eye(2), Q)  # 128x128, W[I,I] for I>=2
    # top-left 2x2 block grid: W[i,k] = R[2i:2i+2, 2k:2k+2] (x) Q
    Wtop = [[np.kron(R[2 * i : 2 * i + 2, 2 * k : 2 * k + 2], Q) for k in range(2)]
            for i in range(2)]
    BT = B.T
    # pass-A packed rhs for I<2:  PA[K] = [ W[0,K]^T | W[1,K]^T ]
    PA0 = np.concatenate([Wtop[0][0].T, Wtop[1][0].T], axis=1)
    PA1 = np.concatenate([Wtop[0][1].T, Wtop[1][1].T], axis=1)
    # pass-B rhs per core:  c<2 -> [ W[0,c]^T | W[1,c]^T ],  c>=2 -> [ B^T | 0 ]
    rhsb = []
    for c in range(NCORES):
        if c < 2:
            rhsb.append(np.concatenate([Wtop[0][c].T, Wtop[1][c].T], axis=1))
        else:
            rhsb.append(np.concatenate([BT, np.zeros_like(BT)], axis=1))
    f32 = np.float32
    return (
        np.ascontiguousarray(BT, dtype=f32),
        np.ascontiguousarray(PA0, dtype=f32),
        np.ascontiguousarray(PA1, dtype=f32),
        [np.ascontiguousarray(r, dtype=f32) for r in rhsb],
    )


# blob columns: BT | RB | rho2..rho7 | PA0 | PA1 | rho0 rho1 — ordered by
# when the kernel needs them; the input DMA is split into a serialized
# chain along this order so early chunks land early (concurrent DMAs
# round-robin at packet granularity and would all finish together)
BLOB_W = 1920
_BT_C, _RB_C, _RHO27_C, _PA_C, _RHO01_C = 0, 128, 384, 1152, 1664
# chain chunk boundaries (columns)
_CHAIN = [0, 384, 640, 896, 1152, 1664, 1920]


def _build_program():
    f32 = mybir.dt.float32
    nc = bacc.Bacc(None)
    blob_in = nc.declare_dram_parameter("blob", [P, BLOB_W], f32, isOutput=False)
    # partition-major output: outp[p, I*256+n] = out[I*128+p, n] -- keeps
    # DMA descriptors at 8KB instead of 1KB; host un-transposes
    outp = nc.declare_dram_parameter("outp", [P, 8 * 256], f32, isOutput=True)

    with TileContext(nc) as tc:
        with (
            tc.tile_pool(name="const", bufs=1) as cpool,
            tc.tile_pool(name="psum", bufs=2, space="PSUM") as ppool,
        ):
            blob = cpool.tile([P, BLOB_W], f32)
            # SWDGE (gpsimd) is pinned to one queue -> chunks complete in
            # issue order at full rate, so early chunks unblock compute
            # early without artificial serialization
            for lo, hi in zip(_CHAIN[:-1], _CHAIN[1:]):
                nc.gpsimd.dma_start(blob[:, lo:hi], blob_in[:, lo:hi])

            zt = cpool.tile([P, 8, P], f32)
            rho = {}
            for pos, K in enumerate([2, 3, 4, 5, 6, 7]):
                rho[K] = blob[:, _RHO27_C + pos * P : _RHO27_C + (pos + 1) * P]
            rho[0] = blob[:, _RHO01_C : _RHO01_C + P]
            rho[1] = blob[:, _RHO01_C + P : _RHO01_C + 2 * P]
            BT = blob[:, _BT_C : _BT_C + 128]
            PA = [blob[:, _PA_C : _PA_C + 256], blob[:, _PA_C + 256 : _PA_C + 512]]
            rb = blob[:, _RB_C : _RB_C + 256]

            def pass_b(I):
                ps = ppool.tile([P, 256], f32, tag="psB")
                nc.tensor.matmul(ps[:], zt[:, I, :], rb, start=True, stop=True)
                ob = cpool.tile([P, 256], f32, tag=f"ob{I}")
                nc.vector.tensor_copy(out=ob[:], in_=ps[:])
                nc.sync.dma_start(outp[:, I * 256 : (I + 1) * 256], ob[:])

            # interleave pass A / pass B per block: B(I) only needs ZT[c,I]
            for I in range(2, 8):
                ps = ppool.tile([P, P], f32, tag="psA")
                nc.tensor.matmul(ps[:], rho[I], BT, start=True, stop=True)
                nc.vector.tensor_copy(out=zt[:, I, :], in_=ps[:])
                pass_b(I)
            ps2 = ppool.tile([P, 256], f32, tag="psA2")
            nc.tensor.matmul(ps2[:], rho[0], PA[0], start=True, stop=False)
            nc.tensor.matmul(ps2[:], rho[1], PA[1], start=False, stop=True)
            nc.vector.tensor_copy(out=zt[:, 0, :], in_=ps2[:, 0:128])
            nc.vector.tensor_copy(out=zt[:, 1, :], in_=ps2[:, 128:256])
            pass_b(0)
            pass_b(1)
    if not nc.is_finalized():
        nc.finalize()
    return nc


def _run(inputs, trace=False):
    global LAST_EXEC_NS
    rho = np.ascontiguousarray(np.asarray(inputs["input_state"], dtype=np.float32))
    thetas = np.asarray(inputs["thetas"], dtype=np.float32)
    BT, PA0, PA1, rhsb = _build_w_blocks(thetas)

    nc = _build_program()
    in_maps = []
    for c in range(NCORES):
        slab = rho[:, c * P : (c + 1) * P].reshape(8, P, P)
        km = lambda ks: np.transpose(slab[ks], (1, 0, 2)).reshape(P, -1)
        # layout: BT | RB | rho2..7 | PA0 PA1 | rho0 rho1
        blob = np.concatenate(
            [BT, rhsb[c], km([2, 3, 4, 5, 6, 7]), PA0, PA1, km([0, 1])], axis=1
        )
        in_maps.append({"blob": np.ascontiguousarray(blob)})
    res = bass_utils.run_bass_kernel_spmd(
        nc, in_maps, list(range(NCORES)), trace=trace
    )
    LAST_EXEC_NS = res.exec_time_ns

    out = np.empty((D, D), dtype=np.float32)

    def unpack(c):
        # outp[p, I*256+n] -> [1024, 256]
        return (
            res.results[c]["outp"]
            .reshape(P, 8, 256)
            .transpose(1, 0, 2)
            .reshape(D, 256)
        )

    for c in range(2, NCORES):
        out[:, c * P : (c + 1) * P] = unpack(c)[:, :P]
    out[:, 0:256] = unpack(0) + unpack(1)
    return out


def kernel(**inputs):
    return _run(inputs)


# revision 21
# speedup vs baseline: 1.2794x; 1.0017x over previous
"""Trainium2 Bass kernel for the Conv-RBS density-matrix problem.

Math: the reference applies 18 RBS (Givens) gates sequentially as
rho <- U rho U^T.  Conjugations compose, and every gate factorizes over
the (row=16, col=16, ch=4) tensor-product structure of the 1024-dim
space, so the whole scan collapses to

    out = W @ rho @ W.T,   W = R (x) C (x) H

with R, C 16x16 rotations that are identity outside their top-left 4x4
block, and H a dense 4x4 rotation.  In 128x128 blocks W is
block-diagonal with B = I2 (x) C (x) H everywhere except a 2x2 block
grid in the top-left 256x256 corner (R's 4x4 block).

Device scheme (per core c of 8, column-sharded on rho):
  pass A:  ZT[c,I] = sum_K  rho[K,c]^T @ W[I,K]^T     (TensorE: lhsT=rho
           block, rhs=const) -- i.e. the c-th block-row of (W rho)^T.
  pass B:  out[I,J] += ZT[c,I]^T @ W[J,c]^T           (lhsT=ZT block)
Cores c>=2 produce the finished column slab out[:,c]; cores 0,1 produce
partial sums for columns 0..255 which the host adds.  No transposes and
no collectives are needed anywhere.
"""

import contextlib
import ctypes
import os
import sys
import types

import numpy as np

import concourse.bass as bass
import concourse.mybir as mybir
from concourse import bacc, bass_utils
from concourse.tile import TileContext
from concourse.tile_rust import add_dep_helper


def _install_axon_ntff_shim():
    """The agent image's ``antenv`` lacks ``axon_hooks``, so bass_utils'
    trace=True path crashes instead of profiling.  Recreate the hook the
    way trn_boot would have: ctypes into libaxon_pjrt.so."""
    try:
        import antenv.axon_hooks  # noqa: F401
        return
    except ImportError:
        pass
    so_path = "/opt/axon/libaxon_pjrt.so"
    hook = None
    if os.path.exists(so_path):
        try:
            lib = ctypes.CDLL(so_path)
            lib.axon_start_nrt_profile.argtypes = [
                ctypes.POINTER(ctypes.c_int64),
                ctypes.c_size_t,
            ]
            lib.axon_start_nrt_profile.restype = ctypes.c_int64
            lib.axon_stop_nrt_profile.argtypes = [ctypes.c_char_p]
            lib.axon_stop_nrt_profile.restype = ctypes.c_int64

            @contextlib.contextmanager
            def _hook(output_dir, device_ids):
                import jax

                jax.devices()
                if device_ids:
                    ids = (ctypes.c_int64 * len(device_ids))(*device_ids)
                    rc = lib.axon_start_nrt_profile(ids, len(device_ids))
                else:
                    rc = lib.axon_start_nrt_profile(None, 0)
                if rc != 0:
                    raise RuntimeError(f"axon_start_nrt_profile rc={rc}")
                try:
                    yield
                finally:
                    n = lib.axon_stop_nrt_profile(str(output_dir).encode())
                    if n < 0:
                        raise RuntimeError(f"axon_stop_nrt_profile rc={n}")

            hook = _hook
        except (OSError, AttributeError):
            hook = None

    mod = types.ModuleType("antenv.axon_hooks")
    mod.get_axon_ntff_profile_hook = lambda: hook
    mod.set_axon_ntff_profile_hook = lambda h: None
    sys.modules["antenv.axon_hooks"] = mod


def _patch_upload_artifacts():
    """Artifact upload needs bucket creds this container may not have;
    don't let a failed upload kill the profiled run."""
    orig = bass_utils.upload_artifacts
    if getattr(orig, "_safe_wrapped", False):
        return

    def safe_upload(tmpdir):
        try:
            return orig(tmpdir)
        except Exception:
            return tmpdir

    safe_upload._safe_wrapped = True
    bass_utils.upload_artifacts = safe_upload


_install_axon_ntff_shim()
_patch_upload_artifacts()

I_DIM, J_DIM, KGATE = 16, 4, 4
D = 1024
P = 128
NCORES = 8

LAST_EXEC_NS = None  # filled when BASS_TRACE is set


def _gate_list():
    gates = []
    for i in range(KGATE):
        for j in range(i + 1, KGATE):
            gates.append(("row", i, j))
    for i in range(KGATE):
        for j in range(i + 1, KGATE):
            gates.append(("col", i, j))
    for i in range(J_DIM):
        for j in range(i + 1, J_DIM):
            gates.append(("ch", i, j))
    return gates


def _build_w_blocks(thetas):
    """Return (BT, PA0, PA1, RHSB[8]) fp32 host constants."""
    mats = {"row": np.eye(I_DIM), "col": np.eye(I_DIM), "ch": np.eye(J_DIM)}
    for (reg, a, b), th in zip(_gate_list(), np.asarray(thetas, dtype=np.float64)):
        n = mats[reg].shape[0]
        G = np.eye(n)
        c, s = np.cos(th), np.sin(th)
        G[a, a] = c
        G[b, b] = c
        G[a, b] = s
        G[b, a] = -s
        mats[reg] = G @ mats[reg]
    R, C, H = mats["row"], mats["col"], mats["ch"]
    Q = np.kron(C, H)  # 64x64
    B = np.kron(np.eye(2), Q)  # 128x128, W[I,I] for I>=2
    # top-left 2x2 block grid: W[i,k] = R[2i:2i+2, 2k:2k+2] (x) Q
    Wtop = [[np.kron(R[2 * i : 2 * i + 2, 2 * k : 2 * k + 2], Q) for k in range(2)]
            for i in range(2)]
    BT = B.T
    # pass-A packed rhs for I<2:  PA[K] = [ W[0,K]^T | W[1,K]^T ]
    PA0 = np.concatenate([Wtop[0][0].T, Wtop[1][0].T], axis=1)
    PA1 = np.concatenate([Wtop[0][1].T, Wtop[1][1].T], axis=1)
    # pass-B rhs per core:  c<2 -> [ W[0,c]^T | W[1,c]^T ],  c>=2 -> [ B^T | 0 ]
    rhsb = []
    for c in range(NCORES):
        if c < 2:
            rhsb.append(np.concatenate([Wtop[0][c].T, Wtop[1][c].T], axis=1))
        else:
            rhsb.append(np.concatenate([BT, np.zeros_like(BT)], axis=1))
    f32 = np.float32
    return (
        np.ascontiguousarray(BT, dtype=f32),
        np.ascontiguousarray(PA0, dtype=f32),
        np.ascontiguousarray(PA1, dtype=f32),
        [np.ascontiguousarray(r, dtype=f32) for r in rhsb],
    )


# blob columns: BT | RB | rho2..rho7 | PA0 | PA1 | rho0 rho1 — ordered by
# when the kernel needs them; the input DMA is split into a serialized
# chain along this order so early chunks land early (concurrent DMAs
# round-robin at packet granularity and would all finish together)
BLOB_W = 1920
_BT_C, _RB_C, _RHO27_C, _PA_C, _RHO01_C = 0, 128, 384, 1152, 1664
# chain chunk boundaries (columns)
_CHAIN = [0, 384, 640, 896, 1152, 1664, 1920]


def _build_program():
    f32 = mybir.dt.float32
    nc = bacc.Bacc(None)
    blob_in = nc.declare_dram_parameter("blob", [P, BLOB_W], f32, isOutput=False)
    # partition-major output: outp[p, I*256+n] = out[I*128+p, n] -- keeps
    # DMA descriptors at 8KB instead of 1KB; host un-transposes
    outp = nc.declare_dram_parameter("outp", [P, 8 * 256], f32, isOutput=True)

    with TileContext(nc) as tc:
        with (
            tc.tile_pool(name="const", bufs=1) as cpool,
            tc.tile_pool(name="psum", bufs=2, space="PSUM") as ppool,
        ):
            blob = cpool.tile([P, BLOB_W], f32)
            # depth-2 chained HWDGE chunks: at most two input DMAs in
            # flight, so early chunks finish early (fully concurrent DMAs
            # round-robin and would all finish together) without paying a
            # full completion latency per link (a depth-1 chain does)
            chain = []
            for lo, hi in zip(_CHAIN[:-1], _CHAIN[1:]):
                d = nc.sync.dma_start(blob[:, lo:hi], blob_in[:, lo:hi])
                di = d.ins if hasattr(d, "ins") else d
                if len(chain) >= 2:
                    add_dep_helper(di, chain[-2], reason="stagger input DMA chain")
                chain.append(di)

            zt = cpool.tile([P, 8, P], f32)
            rho = {}
            for pos, K in enumerate([2, 3, 4, 5, 6, 7]):
                rho[K] = blob[:, _RHO27_C + pos * P : _RHO27_C + (pos + 1) * P]
            rho[0] = blob[:, _RHO01_C : _RHO01_C + P]
            rho[1] = blob[:, _RHO01_C + P : _RHO01_C + 2 * P]
            BT = blob[:, _BT_C : _BT_C + 128]
            PA = [blob[:, _PA_C : _PA_C + 256], blob[:, _PA_C + 256 : _PA_C + 512]]
            rb = blob[:, _RB_C : _RB_C + 256]

            def pass_b(I):
                ps = ppool.tile([P, 256], f32, tag="psB")
                nc.tensor.matmul(ps[:], zt[:, I, :], rb, start=True, stop=True)
                ob = cpool.tile([P, 256], f32, tag=f"ob{I}")
                nc.vector.tensor_copy(out=ob[:], in_=ps[:])
                nc.sync.dma_start(outp[:, I * 256 : (I + 1) * 256], ob[:])

            # interleave pass A / pass B per block: B(I) only needs ZT[c,I]
            for I in range(2, 8):
                ps = ppool.tile([P, P], f32, tag="psA")
                nc.tensor.matmul(ps[:], rho[I], BT, start=True, stop=True)
                nc.vector.tensor_copy(out=zt[:, I, :], in_=ps[:])
                pass_b(I)
            ps2 = ppool.tile([P, 256], f32, tag="psA2")
            nc.tensor.matmul(ps2[:], rho[0], PA[0], start=True, stop=False)
            nc.tensor.matmul(ps2[:], rho[1], PA[1], start=False, stop=True)
            nc.vector.tensor_copy(out=zt[:, 0, :], in_=ps2[:, 0:128])
            nc.vector.tensor_copy(out=zt[:, 1, :], in_=ps2[:, 128:256])
            pass_b(0)
            pass_b(1)
    if not nc.is_finalized():
        nc.finalize()
    return nc


def _run(inputs, trace=False):
    global LAST_EXEC_NS
    rho = np.ascontiguousarray(np.asarray(inputs["input_state"], dtype=np.float32))
    thetas = np.asarray(inputs["thetas"], dtype=np.float32)
    BT, PA0, PA1, rhsb = _build_w_blocks(thetas)

    nc = _build_program()
    in_maps = []
    for c in range(NCORES):
        slab = rho[:, c * P : (c + 1) * P].reshape(8, P, P)
        km = lambda ks: np.transpose(slab[ks], (1, 0, 2)).reshape(P, -1)
        # layout: BT | RB | rho2..7 | PA0 PA1 | rho0 rho1
        blob = np.concatenate(
            [BT, rhsb[c], km([2, 3, 4, 5, 6, 7]), PA0, PA1, km([0, 1])], axis=1
        )
        in_maps.append({"blob": np.ascontiguousarray(blob)})
    res = bass_utils.run_bass_kernel_spmd(
        nc, in_maps, list(range(NCORES)), trace=trace
    )
    LAST_EXEC_NS = res.exec_time_ns

    out = np.empty((D, D), dtype=np.float32)

    def unpack(c):
        # outp[p, I*256+n] -> [1024, 256]
        return (
            res.results[c]["outp"]
            .reshape(P, 8, 256)
            .transpose(1, 0, 2)
            .reshape(D, 256)
        )

    for c in range(2, NCORES):
        out[:, c * P : (c + 1) * P] = unpack(c)[:, :P]
    out[:, 0:256] = unpack(0) + unpack(1)
    return out


def kernel(**inputs):
    return _run(inputs)


# revision 23
# speedup vs baseline: 1.3979x; 1.0927x over previous
"""Trainium2 Bass kernel for the Conv-RBS density-matrix problem.

Math: the reference applies 18 RBS (Givens) gates sequentially as
rho <- U rho U^T.  Conjugations compose, and every gate factorizes over
the (row=16, col=16, ch=4) tensor-product structure of the 1024-dim
space, so the whole scan collapses to

    out = W @ rho @ W.T,   W = R (x) C (x) H

with R, C 16x16 rotations that are identity outside their top-left 4x4
block, and H a dense 4x4 rotation.  In 128x128 blocks W is
block-diagonal with B = I2 (x) C (x) H everywhere except a 2x2 block
grid in the top-left 256x256 corner (R's 4x4 block).

Device scheme (per core c of 8, column-sharded on rho):
  pass A:  ZT[c,I] = sum_K  rho[K,c]^T @ W[I,K]^T     (TensorE: lhsT=rho
           block, rhs=const) -- i.e. the c-th block-row of (W rho)^T.
  pass B:  out[I,J] += ZT[c,I]^T @ W[J,c]^T           (lhsT=ZT block)
Cores c>=2 produce the finished column slab out[:,c]; cores 0,1 produce
partial sums for columns 0..255 which the host adds.  No transposes and
no collectives are needed anywhere.
"""

import contextlib
import ctypes
import os
import sys
import types

import numpy as np

import concourse.bass as bass
import concourse.mybir as mybir
from concourse import bacc, bass_utils
from concourse.tile import TileContext
from concourse.tile_rust import add_dep_helper


def _install_axon_ntff_shim():
    """The agent image's ``antenv`` lacks ``axon_hooks``, so bass_utils'
    trace=True path crashes instead of profiling.  Recreate the hook the
    way trn_boot would have: ctypes into libaxon_pjrt.so."""
    try:
        import antenv.axon_hooks  # noqa: F401
        return
    except ImportError:
        pass
    so_path = "/opt/axon/libaxon_pjrt.so"
    hook = None
    if os.path.exists(so_path):
        try:
            lib = ctypes.CDLL(so_path)
            lib.axon_start_nrt_profile.argtypes = [
                ctypes.POINTER(ctypes.c_int64),
                ctypes.c_size_t,
            ]
            lib.axon_start_nrt_profile.restype = ctypes.c_int64
            lib.axon_stop_nrt_profile.argtypes = [ctypes.c_char_p]
            lib.axon_stop_nrt_profile.restype = ctypes.c_int64

            @contextlib.contextmanager
            def _hook(output_dir, device_ids):
                import jax

                jax.devices()
                if device_ids:
                    ids = (ctypes.c_int64 * len(device_ids))(*device_ids)
                    rc = lib.axon_start_nrt_profile(ids, len(device_ids))
                else:
                    rc = lib.axon_start_nrt_profile(None, 0)
                if rc != 0:
                    raise RuntimeError(f"axon_start_nrt_profile rc={rc}")
                try:
                    yield
                finally:
                    n = lib.axon_stop_nrt_profile(str(output_dir).encode())
                    if n < 0:
                        raise RuntimeError(f"axon_stop_nrt_profile rc={n}")

            hook = _hook
        except (OSError, AttributeError):
            hook = None

    mod = types.ModuleType("antenv.axon_hooks")
    mod.get_axon_ntff_profile_hook = lambda: hook
    mod.set_axon_ntff_profile_hook = lambda h: None
    sys.modules["antenv.axon_hooks"] = mod


def _patch_upload_artifacts():
    """Artifact upload needs bucket creds this container may not have;
    don't let a failed upload kill the profiled run."""
    orig = bass_utils.upload_artifacts
    if getattr(orig, "_safe_wrapped", False):
        return

    def safe_upload(tmpdir):
        try:
            return orig(tmpdir)
        except Exception:
            return tmpdir

    safe_upload._safe_wrapped = True
    bass_utils.upload_artifacts = safe_upload


_install_axon_ntff_shim()
_patch_upload_artifacts()

I_DIM, J_DIM, KGATE = 16, 4, 4
D = 1024
P = 128
NCORES = 8

LAST_EXEC_NS = None  # filled when BASS_TRACE is set


def _gate_list():
    gates = []
    for i in range(KGATE):
        for j in range(i + 1, KGATE):
            gates.append(("row", i, j))
    for i in range(KGATE):
        for j in range(i + 1, KGATE):
            gates.append(("col", i, j))
    for i in range(J_DIM):
        for j in range(i + 1, J_DIM):
            gates.append(("ch", i, j))
    return gates


def _build_w_blocks(thetas):
    """Return (BT, PA0, PA1, RHSB[8]) fp32 host constants."""
    mats = {"row": np.eye(I_DIM), "col": np.eye(I_DIM), "ch": np.eye(J_DIM)}
    for (reg, a, b), th in zip(_gate_list(), np.asarray(thetas, dtype=np.float64)):
        n = mats[reg].shape[0]
        G = np.eye(n)
        c, s = np.cos(th), np.sin(th)
        G[a, a] = c
        G[b, b] = c
        G[a, b] = s
        G[b, a] = -s
        mats[reg] = G @ mats[reg]
    R, C, H = mats["row"], mats["col"], mats["ch"]
    Q = np.kron(C, H)  # 64x64
    B = np.kron(np.eye(2), Q)  # 128x128, W[I,I] for I>=2
    # top-left 2x2 block grid: W[i,k] = R[2i:2i+2, 2k:2k+2] (x) Q
    Wtop = [[np.kron(R[2 * i : 2 * i + 2, 2 * k : 2 * k + 2], Q) for k in range(2)]
            for i in range(2)]
    BT = B.T
    # pass-A packed rhs for I<2:  PA[K] = [ W[0,K]^T | W[1,K]^T ]
    PA0 = np.concatenate([Wtop[0][0].T, Wtop[1][0].T], axis=1)
    PA1 = np.concatenate([Wtop[0][1].T, Wtop[1][1].T], axis=1)
    # pass-B rhs per core:  c<2 -> [ W[0,c]^T | W[1,c]^T ],  c>=2 -> [ B^T | 0 ]
    rhsb = []
    for c in range(NCORES):
        if c < 2:
            rhsb.append(np.concatenate([Wtop[0][c].T, Wtop[1][c].T], axis=1))
        else:
            rhsb.append(np.concatenate([BT, np.zeros_like(BT)], axis=1))
    f32 = np.float32
    return (
        np.ascontiguousarray(BT, dtype=f32),
        np.ascontiguousarray(PA0, dtype=f32),
        np.ascontiguousarray(PA1, dtype=f32),
        [np.ascontiguousarray(r, dtype=f32) for r in rhsb],
    )


# blob columns: BT | RB | rho2..rho7 | PA0 | PA1 | rho0 rho1 — ordered by
# when the kernel needs them; the input DMA is split into a serialized
# chain along this order so early chunks land early (concurrent DMAs
# round-robin at packet granularity and would all finish together)
BLOB_W = 1920
_BT_C, _RB_C, _RHO27_C, _PA_C, _RHO01_C = 0, 128, 384, 1152, 1664
# chain chunk boundaries (columns)
_CHAIN = [0, 384, 640, 896, 1152, 1664, 1920]


def _build_program_raw():
    """Raw (no TileContext) pipeline with manual semaphores.

    Tile's kernel tail (drain + two all-engine barriers + sem recycling)
    costs ~8-9us on a ~25us kernel; with manual sync the program just
    ends.  Sem init is done by an explicit clear + NRT pseudo barrier,
    mirroring what Bass.__init__ does under target_bir_lowering.
    """
    f32 = mybir.dt.float32
    nc = bacc.Bacc(None)
    blob_in = nc.declare_dram_parameter("blob", [P, BLOB_W], f32, isOutput=False)
    outp = nc.declare_dram_parameter("outp", [P, 8 * 256], f32, isOutput=True)

    s_in = [nc.alloc_semaphore(f"s_in{i}") for i in range(6)]
    s_pe = nc.alloc_semaphore("s_pe")
    s_v = nc.alloc_semaphore("s_v")
    s_out = nc.alloc_semaphore("s_out")
    nums = sorted(h.num for h in [*s_in, s_pe, s_v, s_out])
    sem_range = range(nums[0], nums[-1] + 1)
    assert len(nums) == len(sem_range), "sem ids not contiguous"
    nc.gpsimd.dma_reset(sem_range)
    nc.gpsimd.sem_clear(sem_range)
    nc._nrt_pseudo_barrier()

    blob = nc.alloc_sbuf_tensor("blob_sb", [P, BLOB_W], f32)
    zt = nc.alloc_sbuf_tensor("zt_sb", [P, 8 * P], f32)
    obuf = nc.alloc_sbuf_tensor("obuf_sb", [P, 2048], f32)
    pA = [nc.alloc_psum_tensor(f"pA{i}", [P, P], f32) for i in range(2)]
    pP = nc.alloc_psum_tensor("pP", [P, 256], f32)
    pB = [nc.alloc_psum_tensor(f"pB{i}", [P, 256], f32) for i in range(2)]

    rho = {}
    for pos, K in enumerate([2, 3, 4, 5, 6, 7]):
        rho[K] = blob[:, _RHO27_C + pos * P : _RHO27_C + (pos + 1) * P]
    rho[0] = blob[:, _RHO01_C : _RHO01_C + P]
    rho[1] = blob[:, _RHO01_C + P : _RHO01_C + 2 * P]
    BT = blob[:, _BT_C : _BT_C + 128]
    PA = [blob[:, _PA_C : _PA_C + 256], blob[:, _PA_C + 256 : _PA_C + 512]]
    rb = blob[:, _RB_C : _RB_C + 256]
    ztb = lambda I: zt[:, I * P : (I + 1) * P]

    # --- sync engine: staggered input chunk DMAs (depth 2) ---
    for i, (lo, hi) in enumerate(zip(_CHAIN[:-1], _CHAIN[1:])):
        if i >= 2:
            nc.sync.wait_ge(s_in[i - 2], 16)
        nc.sync.dma_start(blob[:, lo:hi], blob_in[:, lo:hi]).then_inc(s_in[i], 16)

    # --- tensor engine: 16 matmuls, banks rotate A0/A1 and B0/B1 ---
    # (mm_idx, kind). pass A I>=2 -> (rho[I], BT) in pA[.]; pass B I ->
    # (zt[I], rb) in pB[.]; packed pass A -> two accumulating matmuls in pP.
    def pe_wait(sem, v):
        nc.tensor.wait_ge(sem, v)

    pe_seq = []  # (psum_ap, lhsT, rhs, waits, start, stop)
    pe_seq.append((pA[0][:], rho[2], BT, [(s_in[0], 16), (s_in[1], 16)], True, True))
    pe_seq.append((pA[1][:], rho[3], BT, [], True, True))
    pe_seq.append((pB[0][:], ztb(2), rb, [(s_v, 1)], True, True))
    pe_seq.append((pA[0][:], rho[4], BT, [(s_in[2], 16)], True, True))
    pe_seq.append((pB[1][:], ztb(3), rb, [(s_v, 2)], True, True))
    pe_seq.append((pA[1][:], rho[5], BT, [], True, True))
    pe_seq.append((pB[0][:], ztb(4), rb, [(s_v, 4)], True, True))
    pe_seq.append((pA[0][:], rho[6], BT, [(s_in[3], 16)], True, True))
    pe_seq.append((pB[1][:], ztb(5), rb, [(s_v, 6)], True, True))
    pe_seq.append((pA[1][:], rho[7], BT, [], True, True))
    pe_seq.append((pB[0][:], ztb(6), rb, [(s_v, 8)], True, True))
    pe_seq.append((pP[:], rho[0], PA[0], [(s_in[4], 16), (s_in[5], 16)], True, False))
    pe_seq.append((pP[:], rho[1], PA[1], [], False, True))
    pe_seq.append((pB[1][:], ztb(7), rb, [(s_v, 10)], True, True))
    pe_seq.append((pB[0][:], ztb(0), rb, [(s_v, 12)], True, True))
    pe_seq.append((pB[1][:], ztb(1), rb, [(s_v, 14)], True, True))
    for idx, (ps, lhsT, rhs, waits, st, sp) in enumerate(pe_seq):
        for sem, v in waits:
            pe_wait(sem, v)
        nc.tensor.matmul(ps, lhsT, rhs, start=st, stop=sp).then_inc(s_pe, 1)

    # --- vector engine: psum -> sbuf copies in PE product order ---
    ob = lambda I: obuf[:, I * 256 : (I + 1) * 256]
    v_seq = [
        (pA[0][:], ztb(2), 1),
        (pA[1][:], ztb(3), 2),
        (pB[0][:], ob(2), 3),
        (pA[0][:], ztb(4), 4),
        (pB[1][:], ob(3), 5),
        (pA[1][:], ztb(5), 6),
        (pB[0][:], ob(4), 7),
        (pA[0][:], ztb(6), 8),
        (pB[1][:], ob(5), 9),
        (pA[1][:], ztb(7), 10),
        (pB[0][:], ob(6), 11),
        (pP[:, 0:128], ztb(0), 13),
        (pP[:, 128:256], ztb(1), 13),
        (pB[1][:], ob(7), 14),
        (pB[0][:], ob(0), 15),
        (pB[1][:], ob(1), 16),
    ]
    for src, dst, pe_v in v_seq:
        nc.vector.wait_ge(s_pe, pe_v)
        nc.vector.tensor_copy(out=dst, in_=src).then_inc(s_v, 1)

    # --- scalar engine: output DMAs (own HWDGE ring), pairs of blocks ---
    # obuf block I holds out[I]; pairs in block order (2,3),(4,5),(6,7),(0,1)
    out_pairs = [((2, 3), 5), ((4, 5), 9), ((6, 7), 14), ((0, 1), 16)]
    for (i0, i1), v_req in out_pairs:
        nc.scalar.wait_ge(s_v, v_req)
        assert i1 == i0 + 1
        nc.scalar.dma_start(
            outp[:, i0 * 256 : (i1 + 1) * 256], obuf[:, i0 * 256 : (i1 + 1) * 256]
        ).then_inc(s_out, 16)
    nc.scalar.wait_ge(s_out, 64)

    if not nc.is_finalized():
        nc.finalize()
    return nc


def _build_program():
    f32 = mybir.dt.float32
    nc = bacc.Bacc(None)
    blob_in = nc.declare_dram_parameter("blob", [P, BLOB_W], f32, isOutput=False)
    # partition-major output: outp[p, I*256+n] = out[I*128+p, n] -- keeps
    # DMA descriptors at 8KB instead of 1KB; host un-transposes
    outp = nc.declare_dram_parameter("outp", [P, 8 * 256], f32, isOutput=True)

    with TileContext(nc) as tc:
        with (
            tc.tile_pool(name="const", bufs=1) as cpool,
            tc.tile_pool(name="psum", bufs=2, space="PSUM") as ppool,
        ):
            blob = cpool.tile([P, BLOB_W], f32)
            # depth-2 chained HWDGE chunks: at most two input DMAs in
            # flight, so early chunks finish early (fully concurrent DMAs
            # round-robin and would all finish together) without paying a
            # full completion latency per link (a depth-1 chain does)
            chain = []
            for lo, hi in zip(_CHAIN[:-1], _CHAIN[1:]):
                d = nc.sync.dma_start(blob[:, lo:hi], blob_in[:, lo:hi])
                di = d.ins if hasattr(d, "ins") else d
                if len(chain) >= 2:
                    add_dep_helper(di, chain[-2], reason="stagger input DMA chain")
                chain.append(di)

            zt = cpool.tile([P, 8, P], f32)
            rho = {}
            for pos, K in enumerate([2, 3, 4, 5, 6, 7]):
                rho[K] = blob[:, _RHO27_C + pos * P : _RHO27_C + (pos + 1) * P]
            rho[0] = blob[:, _RHO01_C : _RHO01_C + P]
            rho[1] = blob[:, _RHO01_C + P : _RHO01_C + 2 * P]
            BT = blob[:, _BT_C : _BT_C + 128]
            PA = [blob[:, _PA_C : _PA_C + 256], blob[:, _PA_C + 256 : _PA_C + 512]]
            rb = blob[:, _RB_C : _RB_C + 256]

            def pass_b(I):
                ps = ppool.tile([P, 256], f32, tag="psB")
                nc.tensor.matmul(ps[:], zt[:, I, :], rb, start=True, stop=True)
                ob = cpool.tile([P, 256], f32, tag=f"ob{I}")
                nc.vector.tensor_copy(out=ob[:], in_=ps[:])
                nc.sync.dma_start(outp[:, I * 256 : (I + 1) * 256], ob[:])

            # interleave pass A / pass B per block: B(I) only needs ZT[c,I]
            for I in range(2, 8):
                ps = ppool.tile([P, P], f32, tag="psA")
                nc.tensor.matmul(ps[:], rho[I], BT, start=True, stop=True)
                nc.vector.tensor_copy(out=zt[:, I, :], in_=ps[:])
                pass_b(I)
            ps2 = ppool.tile([P, 256], f32, tag="psA2")
            nc.tensor.matmul(ps2[:], rho[0], PA[0], start=True, stop=False)
            nc.tensor.matmul(ps2[:], rho[1], PA[1], start=False, stop=True)
            nc.vector.tensor_copy(out=zt[:, 0, :], in_=ps2[:, 0:128])
            nc.vector.tensor_copy(out=zt[:, 1, :], in_=ps2[:, 128:256])
            pass_b(0)
            pass_b(1)
    if not nc.is_finalized():
        nc.finalize()
    return nc


def _run(inputs, trace=False):
    global LAST_EXEC_NS
    rho = np.ascontiguousarray(np.asarray(inputs["input_state"], dtype=np.float32))
    thetas = np.asarray(inputs["thetas"], dtype=np.float32)
    BT, PA0, PA1, rhsb = _build_w_blocks(thetas)

    nc = _build_program_raw() if os.environ.get("RBS_TILE") != "1" else _build_program()
    in_maps = []
    for c in range(NCORES):
        slab = rho[:, c * P : (c + 1) * P].reshape(8, P, P)
        km = lambda ks: np.transpose(slab[ks], (1, 0, 2)).reshape(P, -1)
        # layout: BT | RB | rho2..7 | PA0 PA1 | rho0 rho1
        blob = np.concatenate(
            [BT, rhsb[c], km([2, 3, 4, 5, 6, 7]), PA0, PA1, km([0, 1])], axis=1
        )
        in_maps.append({"blob": np.ascontiguousarray(blob)})
    res = bass_utils.run_bass_kernel_spmd(
        nc, in_maps, list(range(NCORES)), trace=trace
    )
    LAST_EXEC_NS = res.exec_time_ns

    out = np.empty((D, D), dtype=np.float32)

    def unpack(c):
        # outp[p, I*256+n] -> [1024, 256]
        return (
            res.results[c]["outp"]
            .reshape(P, 8, 256)
            .transpose(1, 0, 2)
            .reshape(D, 256)
        )

    for c in range(2, NCORES):
        out[:, c * P : (c + 1) * P] = unpack(c)[:, :P]
    out[:, 0:256] = unpack(0) + unpack(1)
    return out


def kernel(**inputs):
    return _run(inputs)


# revision 27
# speedup vs baseline: 1.4158x; 1.0128x over previous
"""Trainium2 Bass kernel for the Conv-RBS density-matrix problem.

Math: the reference applies 18 RBS (Givens) gates sequentially as
rho <- U rho U^T.  Conjugations compose, and every gate factorizes over
the (row=16, col=16, ch=4) tensor-product structure of the 1024-dim
space, so the whole scan collapses to

    out = W @ rho @ W.T,   W = R (x) C (x) H

with R, C 16x16 rotations that are identity outside their top-left 4x4
block, and H a dense 4x4 rotation.  In 128x128 blocks W is
block-diagonal with B = I2 (x) C (x) H everywhere except a 2x2 block
grid in the top-left 256x256 corner (R's 4x4 block).

Device scheme (per core c of 8, column-sharded on rho):
  pass A:  ZT[c,I] = sum_K  rho[K,c]^T @ W[I,K]^T     (TensorE: lhsT=rho
           block, rhs=const) -- i.e. the c-th block-row of (W rho)^T.
  pass B:  out[I,J] += ZT[c,I]^T @ W[J,c]^T           (lhsT=ZT block)
Cores c>=2 produce the finished column slab out[:,c]; cores 0,1 produce
partial sums for columns 0..255 which the host adds.  No transposes and
no collectives are needed anywhere.
"""

import contextlib
import ctypes
import os
import sys
import types

import numpy as np

import concourse.bass as bass
import concourse.mybir as mybir
from concourse import bacc, bass_utils


def _install_axon_ntff_shim():
    """The agent image's ``antenv`` lacks ``axon_hooks``, so bass_utils'
    trace=True path crashes instead of profiling.  Recreate the hook the
    way trn_boot would have: ctypes into libaxon_pjrt.so."""
    try:
        import antenv.axon_hooks  # noqa: F401
        return
    except ImportError:
        pass
    so_path = "/opt/axon/libaxon_pjrt.so"
    hook = None
    if os.path.exists(so_path):
        try:
            lib = ctypes.CDLL(so_path)
            lib.axon_start_nrt_profile.argtypes = [
                ctypes.POINTER(ctypes.c_int64),
                ctypes.c_size_t,
            ]
            lib.axon_start_nrt_profile.restype = ctypes.c_int64
            lib.axon_stop_nrt_profile.argtypes = [ctypes.c_char_p]
            lib.axon_stop_nrt_profile.restype = ctypes.c_int64

            @contextlib.contextmanager
            def _hook(output_dir, device_ids):
                import jax

                jax.devices()
                if device_ids:
                    ids = (ctypes.c_int64 * len(device_ids))(*device_ids)
                    rc = lib.axon_start_nrt_profile(ids, len(device_ids))
                else:
                    rc = lib.axon_start_nrt_profile(None, 0)
                if rc != 0:
                    raise RuntimeError(f"axon_start_nrt_profile rc={rc}")
                try:
                    yield
                finally:
                    n = lib.axon_stop_nrt_profile(str(output_dir).encode())
                    if n < 0:
                        raise RuntimeError(f"axon_stop_nrt_profile rc={n}")

            hook = _hook
        except (OSError, AttributeError):
            hook = None

    mod = types.ModuleType("antenv.axon_hooks")
    mod.get_axon_ntff_profile_hook = lambda: hook
    mod.set_axon_ntff_profile_hook = lambda h: None
    sys.modules["antenv.axon_hooks"] = mod


def _patch_upload_artifacts():
    """Artifact upload needs bucket creds this container may not have;
    don't let a failed upload kill the profiled run."""
    orig = bass_utils.upload_artifacts
    if getattr(orig, "_safe_wrapped", False):
        return

    def safe_upload(tmpdir):
        try:
            return orig(tmpdir)
        except Exception:
            return tmpdir

    safe_upload._safe_wrapped = True
    bass_utils.upload_artifacts = safe_upload


_install_axon_ntff_shim()
_patch_upload_artifacts()

I_DIM, J_DIM, KGATE = 16, 4, 4
D = 1024
P = 128
NCORES = 8

LAST_EXEC_NS = None  # filled when BASS_TRACE is set


def _gate_list():
    gates = []
    for i in range(KGATE):
        for j in range(i + 1, KGATE):
            gates.append(("row", i, j))
    for i in range(KGATE):
        for j in range(i + 1, KGATE):
            gates.append(("col", i, j))
    for i in range(J_DIM):
        for j in range(i + 1, J_DIM):
            gates.append(("ch", i, j))
    return gates


def _build_w_blocks(thetas):
    """Return (BT, PA0, PA1, RHSB[8]) fp32 host constants."""
    mats = {"row": np.eye(I_DIM), "col": np.eye(I_DIM), "ch": np.eye(J_DIM)}
    for (reg, a, b), th in zip(_gate_list(), np.asarray(thetas, dtype=np.float64)):
        n = mats[reg].shape[0]
        G = np.eye(n)
        c, s = np.cos(th), np.sin(th)
        G[a, a] = c
        G[b, b] = c
        G[a, b] = s
        G[b, a] = -s
        mats[reg] = G @ mats[reg]
    R, C, H = mats["row"], mats["col"], mats["ch"]
    Q = np.kron(C, H)  # 64x64
    B = np.kron(np.eye(2), Q)  # 128x128, W[I,I] for I>=2
    # top-left 2x2 block grid: W[i,k] = R[2i:2i+2, 2k:2k+2] (x) Q
    Wtop = [[np.kron(R[2 * i : 2 * i + 2, 2 * k : 2 * k + 2], Q) for k in range(2)]
            for i in range(2)]
    BT = B.T
    # pass-A packed rhs for I<2:  PA[K] = [ W[0,K]^T | W[1,K]^T ]
    PA0 = np.concatenate([Wtop[0][0].T, Wtop[1][0].T], axis=1)
    PA1 = np.concatenate([Wtop[0][1].T, Wtop[1][1].T], axis=1)
    # pass-B rhs per core:  c<2 -> [ W[0,c]^T | W[1,c]^T ],  c>=2 -> [ B^T | 0 ]
    rhsb = []
    for c in range(NCORES):
        if c < 2:
            rhsb.append(np.concatenate([Wtop[0][c].T, Wtop[1][c].T], axis=1))
        else:
            rhsb.append(np.concatenate([BT, np.zeros_like(BT)], axis=1))
    f32 = np.float32
    return (
        np.ascontiguousarray(BT, dtype=f32),
        np.ascontiguousarray(PA0, dtype=f32),
        np.ascontiguousarray(PA1, dtype=f32),
        [np.ascontiguousarray(r, dtype=f32) for r in rhsb],
    )


# blob layout: columns ordered by when the kernel needs them; the input
# DMA is a depth-2 staggered chain of chunks so early columns land early
# (fully concurrent DMAs round-robin and would all finish together).
# segments: BT RB rho2 | rho3 rho4 | rho5 rho6 | rho7 | PA0 PA1 | rho0 rho1
BLOB_W = 1920
_BT_C, _RB_C = 0, 128
_RHO_C = {2: 384, 3: 512, 4: 640, 5: 768, 6: 896, 7: 1024, 0: 1664, 1: 1792}
_PA_C = 1152
_CHAIN = [0, 512, 768, 1024, 1152, 1664, 1920]


def _build_program_raw():
    """Raw (no TileContext) pipeline with manual semaphores.

    Tile's kernel tail (drain + two all-engine barriers + sem recycling)
    costs ~8-9us on a ~25us kernel; with manual sync the program just
    ends.  Sem init is done by an explicit clear + NRT pseudo barrier,
    mirroring what Bass.__init__ does under target_bir_lowering.
    """
    f32 = mybir.dt.float32
    nc = bacc.Bacc(None)
    blob_in = nc.declare_dram_parameter("blob", [P, BLOB_W], f32, isOutput=False)
    outp = nc.declare_dram_parameter("outp", [P, 8 * 256], f32, isOutput=True)

    s_in = [nc.alloc_semaphore(f"s_in{i}") for i in range(6)]
    s_pe = nc.alloc_semaphore("s_pe")
    s_v = nc.alloc_semaphore("s_v")
    s_out = nc.alloc_semaphore("s_out")
    nums = sorted(h.num for h in [*s_in, s_pe, s_v, s_out])
    sem_range = range(nums[0], nums[-1] + 1)
    assert len(nums) == len(sem_range), "sem ids not contiguous"
    nc.gpsimd.dma_reset(sem_range)
    nc.gpsimd.sem_clear(sem_range)
    nc._nrt_pseudo_barrier()

    blob = nc.alloc_sbuf_tensor("blob_sb", [P, BLOB_W], f32)
    zt = nc.alloc_sbuf_tensor("zt_sb", [P, 8 * P], f32)
    obuf = nc.alloc_sbuf_tensor("obuf_sb", [P, 2048], f32)
    pA = [nc.alloc_psum_tensor(f"pA{i}", [P, P], f32) for i in range(2)]
    pP = nc.alloc_psum_tensor("pP", [P, 256], f32)
    pB = [nc.alloc_psum_tensor(f"pB{i}", [P, 256], f32) for i in range(2)]

    rho = {K: blob[:, c : c + P] for K, c in _RHO_C.items()}
    BT = blob[:, _BT_C : _BT_C + 128]
    PA = [blob[:, _PA_C : _PA_C + 256], blob[:, _PA_C + 256 : _PA_C + 512]]
    rb = blob[:, _RB_C : _RB_C + 256]
    ztb = lambda I: zt[:, I * P : (I + 1) * P]

    # --- sync engine: staggered input chunk DMAs (depth 2) ---
    for i, (lo, hi) in enumerate(zip(_CHAIN[:-1], _CHAIN[1:])):
        if i >= 2:
            nc.sync.wait_ge(s_in[i - 2], 16)
        nc.sync.dma_start(blob[:, lo:hi], blob_in[:, lo:hi]).then_inc(s_in[i], 16)

    # --- tensor engine: 16 matmuls, banks rotate A0/A1 and B0/B1 ---
    # pass A I>=2 -> (rho[I], BT) in pA[.]; pass B I -> (zt[I], rb) in
    # pB[.]; packed pass A -> two accumulating matmuls in pP.
    pe_seq = []  # (psum_ap, lhsT, rhs, waits, start, stop)
    pe_seq.append((pA[0][:], rho[2], BT, [(s_in[0], 16)], True, True))
    pe_seq.append((pA[1][:], rho[3], BT, [(s_in[1], 16)], True, True))
    pe_seq.append((pB[0][:], ztb(2), rb, [(s_v, 1)], True, True))
    pe_seq.append((pA[0][:], rho[4], BT, [], True, True))
    pe_seq.append((pB[1][:], ztb(3), rb, [(s_v, 2)], True, True))
    pe_seq.append((pA[1][:], rho[5], BT, [(s_in[2], 16)], True, True))
    pe_seq.append((pB[0][:], ztb(4), rb, [(s_v, 4)], True, True))
    pe_seq.append((pA[0][:], rho[6], BT, [], True, True))
    pe_seq.append((pB[1][:], ztb(5), rb, [(s_v, 6)], True, True))
    pe_seq.append((pA[1][:], rho[7], BT, [(s_in[3], 16)], True, True))
    pe_seq.append((pB[0][:], ztb(6), rb, [(s_v, 8)], True, True))
    pe_seq.append((pP[:], rho[0], PA[0], [(s_in[4], 16), (s_in[5], 16)], True, False))
    pe_seq.append((pP[:], rho[1], PA[1], [], False, True))
    pe_seq.append((pB[1][:], ztb(7), rb, [(s_v, 10)], True, True))
    pe_seq.append((pB[0][:], ztb(0), rb, [(s_v, 12)], True, True))
    pe_seq.append((pB[1][:], ztb(1), rb, [(s_v, 14)], True, True))
    for ps, lhsT, rhs, waits, st, sp in pe_seq:
        for sem, v in waits:
            nc.tensor.wait_ge(sem, v)
        nc.tensor.matmul(ps, lhsT, rhs, start=st, stop=sp).then_inc(s_pe, 1)

    # --- vector engine: psum -> sbuf copies in PE product order ---
    ob = lambda I: obuf[:, I * 256 : (I + 1) * 256]
    v_seq = [
        (pA[0][:], ztb(2), 1),
        (pA[1][:], ztb(3), 2),
        (pB[0][:], ob(2), 3),
        (pA[0][:], ztb(4), 4),
        (pB[1][:], ob(3), 5),
        (pA[1][:], ztb(5), 6),
        (pB[0][:], ob(4), 7),
        (pA[0][:], ztb(6), 8),
        (pB[1][:], ob(5), 9),
        (pA[1][:], ztb(7), 10),
        (pB[0][:], ob(6), 11),
        (pP[:, 0:128], ztb(0), 13),
        (pP[:, 128:256], ztb(1), 13),
        (pB[1][:], ob(7), 14),
        (pB[0][:], ob(0), 15),
        (pB[1][:], ob(1), 16),
    ]
    for src, dst, pe_v in v_seq:
        nc.vector.wait_ge(s_pe, pe_v)
        nc.vector.tensor_copy(out=dst, in_=src).then_inc(s_v, 1)

    # --- scalar engine: output DMAs (own HWDGE ring) ---
    # obuf block I holds out[I]; last two go out singly so o(0) doesn't
    # wait for the very last copy
    out_grps = [((2, 3), 5), ((4, 5), 9), ((6, 7), 14), ((0, 0), 15), ((1, 1), 16)]
    for (i0, i1), v_req in out_grps:
        nc.scalar.wait_ge(s_v, v_req)
        nc.scalar.dma_start(
            outp[:, i0 * 256 : (i1 + 1) * 256], obuf[:, i0 * 256 : (i1 + 1) * 256]
        ).then_inc(s_out, 16)
    nc.scalar.wait_ge(s_out, 16 * len(out_grps))

    if not nc.is_finalized():
        nc.finalize()
    return nc


def _run(inputs, trace=False):
    global LAST_EXEC_NS
    rho = np.ascontiguousarray(np.asarray(inputs["input_state"], dtype=np.float32))
    thetas = np.asarray(inputs["thetas"], dtype=np.float32)
    BT, PA0, PA1, rhsb = _build_w_blocks(thetas)

    nc = _build_program_raw()
    in_maps = []
    for c in range(NCORES):
        slab = rho[:, c * P : (c + 1) * P].reshape(8, P, P)
        km = lambda ks: np.transpose(slab[ks], (1, 0, 2)).reshape(P, -1)
        # layout: BT | RB | rho2..7 | PA0 PA1 | rho0 rho1
        blob = np.concatenate(
            [BT, rhsb[c], km([2, 3, 4, 5, 6, 7]), PA0, PA1, km([0, 1])], axis=1
        )
        in_maps.append({"blob": np.ascontiguousarray(blob)})
    res = bass_utils.run_bass_kernel_spmd(
        nc, in_maps, list(range(NCORES)), trace=trace
    )
    LAST_EXEC_NS = res.exec_time_ns

    out = np.empty((D, D), dtype=np.float32)

    def unpack(c):
        # outp[p, I*256+n] -> [1024, 256]
        return (
            res.results[c]["outp"]
            .reshape(P, 8, 256)
            .transpose(1, 0, 2)
            .reshape(D, 256)
        )

    for c in range(2, NCORES):
        out[:, c * P : (c + 1) * P] = unpack(c)[:, :P]
    out[:, 0:256] = unpack(0) + unpack(1)
    return out


def kernel(**inputs):
    return _run(inputs)
